# revision 15
# baseline (speedup 1.0000x reference)
"""Sparse transposed-conv (27-tap gather-GEMM) + BatchNorm + LeakyReLU on 8 TRN2 cores.

Strategy (voxel-sharded, compacted kernel map, SBUF-resident h):
  - Host compacts nbr into per-(core, tap, src-window) valid (src, dst) pair lists
    (~87% of nbr entries are -1 and are dropped entirely).
  - Device, per core: dma_gather valid x rows (int16 idx, windowed source);
    PE-transpose 128x128 chunks into channel-major; one fp32 matmul per 512
    columns against a block-diag [[Wk,0],[0,Wk]] stationary (2 voxels per
    streamed column); PE-transpose back; dma_scatter_add (SBUF parity-split
    CCE mode, tokens_per_rank=128) into two SBUF h accumulators -- no HBM
    round trip for h.
  - Center tap (identity map) runs dense and writes the h buffers directly.
  - BN tail: per-(partition, channel) partials from the SBUF h parity
    layout, one PE-transpose fold, 8-core AllReduce of [sum, sumsq],
    per-column scale/bias broadcast tiles, DVE mul/add + ACT Lrelu in
    place, y DMA'd out per parity.

h parity layout: voxel row v -> buf (v//128)%2, partition v%128,
free cols ((v//256)*64 .. +64). VP=25600 rows; rows 25088..25599 are a
trash zone for scatter padding (spread over 512 rows to avoid a single
RMW hotspot), zeroed before stats.
"""
import os
import numpy as np

import concourse.bass as bass
import concourse.mybir as mybir
import concourse.bacc as bacc
import concourse.tile as tile
from concourse import bass_utils
from concourse.masks import make_identity

N = 200000
C = 8
V = N // C          # 25000 voxels per core
D = 64
K = 27
KC = 13             # center tap (identity map)
VP = 25600          # h rows incl. trash zone
TRASH = 25088       # junk-pad scatter rows: blocks 196..199 only
NTRASH = 512
WIN = 32768
NW = (N + WIN - 1) // WIN   # 7 source windows
EPS = 1e-5
NEG = 0.01
F32 = mybir.dt.float32
I16 = mybir.dt.int16
GPAIR = VP // 256   # 100 column-groups of 64 per h buffer


def _r128(n):
    return (n + 127) & ~127


def _r256(n):
    return (n + 255) & ~255


def _pack16(slab, col0, vals):
    """Place index list (len mult of 16) at int16-slab columns col0.., wrapped
    [i%16, i//16] and replicated to all 8 Q7 core partition groups."""
    w = vals.reshape(-1, 16).T  # [16, L/16]
    L16 = w.shape[1]
    for r in range(8):
        slab[r * 16:(r + 1) * 16, col0:col0 + L16] = w
    return col0 + L16


def _prep_host(nbr):
    """Compact kernel maps. Returns per-k segment plan + per-core idx slabs.

    Per (tap, window) segment: all cores trimmed to the same T (max count);
    positions [count_c, T) are junk pads (gather: idx 0 / scatter: trash
    rows); [T, seg_len) are -1 (dropped by the ucode trailing-trim, so they
    cost no descriptors).
    """
    ks = [k for k in range(K) if k != KC]
    lists = {}
    for c in range(C):
        sl = slice(c * V, (c + 1) * V)
        for k in ks:
            src = nbr[k, sl]
            valid = np.nonzero(src >= 0)[0]
            s = src[valid]
            w_of = s // WIN
            for w in range(NW):
                m = w_of == w
                lists[(c, k, w)] = (
                    (s[m] - w * WIN).astype(np.int16),
                    valid[m].astype(np.int16),
                )
    T = {}
    seg_len = {}
    for k in ks:
        for w in range(NW):
            mx = max(len(lists[(c, k, w)][0]) for c in range(C))
            T[(k, w)] = mx
            seg_len[(k, w)] = _r128(mx)
    NK = {}
    for k in ks:
        tot = sum(seg_len[(k, w)] for w in range(NW))
        tot2 = _r256(tot)
        if tot2 > tot:
            # fold the round-up into the last nonempty window (keeps the
            # per-tap total a multiple of 256 for the 2-voxel matmul chunks;
            # the extra positions are -1 so they cost nothing)
            for w in reversed(range(NW)):
                if seg_len[(k, w)] > 0:
                    seg_len[(k, w)] += tot2 - tot
                    break
        NK[k] = tot2
    GT = sum(NK[k] for k in ks)          # total gather/scatter positions
    gslab = np.zeros((C, 128, GT // 16), np.int16)
    sslab = np.zeros((C, 128, GT // 16), np.int16)
    plan = []   # (k, [(w, seg_len, T)...], koff, NKk, sc_trim)
    koff = 0
    trash_ctr = 0
    for k in ks:
        segs = [(w, seg_len[(k, w)], T[(k, w)])
                for w in range(NW) if seg_len[(k, w)] > 0]
        last_off = sum(L for _, L, _ in segs[:-1])
        sc_trim = last_off + segs[-1][2]
        plan.append((k, segs, koff, NK[k], sc_trim))
        for c in range(C):
            col = koff // 16
            sc = []
            for si, (w, L, Tw) in enumerate(segs):
                g, s = lists[(c, k, w)]
                gp = np.full(L, -1, np.int16)
                gp[:len(g)] = g
                gp[len(g):Tw] = 0          # uniform-trim junk pads
                sp = np.full(L, -1, np.int16)
                sp[:len(s)] = s
                # valid-trash pads up to Tw always; for non-last segments the
                # alignment tail [Tw, L) must also be valid (interior -1 is
                # illegal), only the last segment's tail stays -1
                pad_to = Tw if si == len(segs) - 1 else L
                npad = pad_to - len(s)
                if npad > 0:
                    sp[len(s):pad_to] = TRASH + (
                        (trash_ctr + np.arange(npad)) % NTRASH)
                    trash_ctr += npad
                col = _pack16(gslab[c], col, gp)
                sc.append(sp)
            _pack16(sslab[c], koff // 16, np.concatenate(sc))
        koff += NK[k]
    return plan, NK, GT, gslab, sslab


def _build_program(plan, GT, NKmax):
    nc = bacc.Bacc("TRN2", target_bir_lowering=False, debug=False, num_devices=C)
    _build_body(nc)
    nc.compile()
    return nc


def _build_body(nc):
    plan, GT, NKmax = _CURRENT_PLAN
    x_d = nc.dram_tensor("x_d", [N, D], F32, kind="ExternalInput")
    xc_d = nc.dram_tensor("xc_d", [V, D], F32, kind="ExternalInput")
    W_d = nc.dram_tensor("W_d", [K, D, D], F32, kind="ExternalInput")
    gam_d = nc.dram_tensor("gam_d", [1, D], F32, kind="ExternalInput")
    bet_d = nc.dram_tensor("bet_d", [1, D], F32, kind="ExternalInput")
    gi_d = nc.dram_tensor("gi_d", [128, GT // 16], I16, kind="ExternalInput")
    si_d = nc.dram_tensor("si_d", [128, GT // 16], I16, kind="ExternalInput")
    y_d = nc.dram_tensor("y_d", [V, D], F32, kind="ExternalOutput")

    with tile.TileContext(nc) as tc:
        with tc.tile_pool(name="sb", bufs=1) as sb, \
             tc.tile_pool(name="io", bufs=3) as io, \
             tc.tile_pool(name="ps", bufs=2, space="PSUM") as ps, \
             tc.tile_pool(name="dram", bufs=1, space="DRAM") as dram:

            # SBUF-resident h accumulators (parity-split voxel-major)
            hA = sb.tile([128, GPAIR * D], F32)   # even 128-row blocks
            hB = sb.tile([128, GPAIR * D], F32)   # odd 128-row blocks

            ident = sb.tile([128, 128], F32)
            make_identity(nc, ident[:])
            gi_t = sb.tile([128, GT // 16], I16)
            nc.sync.dma_start(gi_t[:], gi_d[:, :])
            si_t = sb.tile([128, GT // 16], I16)
            nc.sync.dma_start(si_t[:], si_d[:, :])

            W2 = sb.tile([128, K * 128], F32)
            nc.gpsimd.memset(W2[:], 0.0)
            for k in range(K):
                nc.sync.dma_start(W2[0:D, k * 128:k * 128 + D], W_d[k, :, :])
                nc.sync.dma_start(W2[D:128, k * 128 + D:(k + 1) * 128], W_d[k, :, :])

            zt = sb.tile([128, 128], F32)
            nc.gpsimd.memset(zt[:], 0.0)

            def mid(gsrc, kk, cols, sout, scol):
                """gsrc[:, cols] (2-slot chunks) -> transpose -> MM W2[kk]
                -> transpose back -> sout[:, scol:scol+len(cols)*128]."""
                gw = len(cols) * 128
                pa = ps.tile([128, 512], F32, tag="psA", space="PSUM")
                for j, cj in enumerate(cols):
                    nc.tensor.transpose(
                        out=pa[:, j * 128:(j + 1) * 128],
                        in_=gsrc[:, cj * 128:(cj + 1) * 128], identity=ident[:])
                ct = io.tile([128, 512], F32, tag="ct")
                nc.vector.tensor_copy(ct[:, :gw], pa[:, :gw])
                pb = ps.tile([128, 512], F32, tag="psB", space="PSUM")
                nc.tensor.matmul(out=pb[:, :gw],
                                 lhsT=W2[:, kk * 128:(kk + 1) * 128],
                                 rhs=ct[:, :gw], start=True, stop=True)
                hb = io.tile([128, 512], F32, tag="hb")
                nc.vector.tensor_copy(hb[:, :gw], pb[:, :gw])
                pc = ps.tile([128, 512], F32, tag="psC", space="PSUM")
                for j in range(len(cols)):
                    nc.tensor.transpose(
                        out=pc[:, j * 128:(j + 1) * 128],
                        in_=hb[:, j * 128:(j + 1) * 128], identity=ident[:])
                nc.scalar.activation(sout[:, scol:scol + gw], pc[:, :gw],
                                     mybir.ActivationFunctionType.Copy, bias=0.0)

            # ---- center tap: dense, writes h buffers directly ----
            # 512-row groups g: rows 512g + s*128 + p, slot s in 0..3;
            # slot s -> block 4g+s -> buf (s%2), pair col (2g + s//2)
            xcv = xc_d[0:24576, :].rearrange("(g s p) c -> g p s c", s=4, p=128)
            for g in range(49):
                xg = io.tile([128, 256], F32, tag="xg")
                if g < 48:
                    nc.sync.dma_start(
                        xg[:].rearrange("p (s c) -> p s c", s=4), xcv[g])
                else:
                    # tail rows 24576..24999 (424 rows = 3 slots + 40)
                    nc.gpsimd.memset(xg[:], 0.0)
                    nc.sync.dma_start(
                        xg[:, 0:192].rearrange("p (s c) -> p s c", s=3),
                        xc_d[24576:24960, :].rearrange(
                            "(s p) c -> p s c", s=3, p=128))
                    nc.sync.dma_start(
                        xg[0:40, 192:256], xc_d[24960:25000, :])
                so = io.tile([128, 256], F32, tag="so")
                mid(xg, KC, [0, 1], so, 0)
                nc.vector.tensor_copy(hA[:, (2 * g) * D:(2 * g + 1) * D],
                                      so[:, 0:64])
                nc.vector.tensor_copy(hB[:, (2 * g) * D:(2 * g + 1) * D],
                                      so[:, 64:128])
                nc.vector.tensor_copy(hA[:, (2 * g + 1) * D:(2 * g + 2) * D],
                                      so[:, 128:192])
                nc.vector.tensor_copy(hB[:, (2 * g + 1) * D:(2 * g + 2) * D],
                                      so[:, 192:256])

            # zero pairs 98,99 (blocks 196..199, never center-written) so the
            # CCE scatter reads initialized data
            nc.vector.tensor_copy(hA[:, 98 * D:100 * D], zt[:])
            nc.vector.tensor_copy(hB[:, 98 * D:100 * D], zt[:])

            # ---- 26 sparse taps: gather -> GEMM -> SBUF CCE scatter-add ----
            for ki, (k, segs, koff, NKk, sc_trim) in enumerate(plan):
                gb = io.tile([128, NKmax // 128, D], F32, tag="gb")
                # the gather trailing-trim leaves pad rows unwritten; zero them
                # so junk can't poison paired voxels through the block-diag MM
                nc.vector.memset(gb[:], 0.0)
                soff = 0
                for w, L, Tw in segs:
                    wlo = w * WIN
                    whi = min(N, wlo + WIN)
                    nc.gpsimd.dma_gather(
                        out_ap=gb[:, soff // 128:(soff + L) // 128, :],
                        in_ap=x_d[wlo:whi, :],
                        idxs_ap=gi_t[:, (koff + soff) // 16:(koff + soff + L) // 16],
                        num_idxs=L, num_idxs_reg=Tw, elem_size=D,
                        single_packet=False)
                    soff += L
                gbf = gb[:].rearrange("p m d -> p (m d)")
                sk = io.tile([128, NKmax // 128, D], F32, tag="sk")
                skf = sk[:].rearrange("p m d -> p (m d)")
                nch = NKk // 256
                for c0 in range(0, nch, 4):
                    cols = list(range(c0, min(c0 + 4, nch)))
                    mid(gbf, k, cols, skf, c0 * 128)
                nc.gpsimd.dma_scatter_add(
                    out_ap=hA[:], in_ap=sk[:, 0:NKk // 128, :],
                    idxs_ap=si_t[:, koff // 16:(koff + NKk) // 16],
                    num_idxs=NKk, num_idxs_reg=sc_trim,
                    elem_size=D, single_packet=False,
                    sbuf_tokens_per_rank=128, parity_reg=0,
                    out_ap_other=hB[:])

            # ---- zero trash rows (25088..25599, blocks 196..199) before
            # stats; block 195's tail is zero via the center tap's padded input
            nc.vector.tensor_copy(hA[:, 98 * D:100 * D], zt[:])
            nc.vector.tensor_copy(hB[:, 98 * D:100 * D], zt[:])

            # ---- BN stats: per-(partition, channel) partials, PE fold ----
            acc = sb.tile([128, 512], F32)
            qacc = sb.tile([128, 512], F32)
            slices = []
            for b in (hA, hB):
                off = 0
                while off < GPAIR * D:
                    wdt = min(512, GPAIR * D - off)
                    slices.append((b, off, wdt))
                    off += wdt
            first = True
            for b, off, wdt in slices:
                src = b[:, off:off + wdt]
                scr = io.tile([128, 512], F32, tag="scr")
                nc.vector.tensor_tensor(out=scr[:, 0:wdt], in0=src, in1=src,
                                        op=mybir.AluOpType.mult)
                if first:
                    nc.vector.tensor_copy(acc[:, 0:wdt], src)
                    nc.vector.tensor_copy(qacc[:, 0:wdt], scr[:, 0:wdt])
                    first = False
                else:
                    nc.vector.tensor_tensor(out=acc[:, 0:wdt], in0=acc[:, 0:wdt],
                                            in1=src, op=mybir.AluOpType.add)
                    nc.vector.tensor_tensor(out=qacc[:, 0:wdt],
                                            in0=qacc[:, 0:wdt],
                                            in1=scr[:, 0:wdt],
                                            op=mybir.AluOpType.add)
            for tgt in (acc, qacc):
                nc.vector.tensor_tensor(out=tgt[:, 0:256], in0=tgt[:, 0:256],
                                        in1=tgt[:, 256:512], op=mybir.AluOpType.add)
                nc.vector.tensor_tensor(out=tgt[:, 0:128], in0=tgt[:, 0:128],
                                        in1=tgt[:, 128:256], op=mybir.AluOpType.add)
                nc.vector.tensor_tensor(out=tgt[:, 0:64], in0=tgt[:, 0:64],
                                        in1=tgt[:, 64:128], op=mybir.AluOpType.add)
            comb = sb.tile([128, 128], F32)
            nc.vector.tensor_copy(comb[:, 0:64], acc[:, 0:64])
            nc.vector.tensor_copy(comb[:, 64:128], qacc[:, 0:64])
            pt = ps.tile([128, 128], F32, tag="psT", space="PSUM")
            nc.tensor.transpose(out=pt[:], in_=comb[:], identity=ident[:])
            tpc = sb.tile([128, 128], F32)
            nc.vector.tensor_copy(tpc[:], pt[:])
            s128 = sb.tile([128, 1], F32)
            nc.vector.tensor_reduce(out=s128[:], in_=tpc[:],
                                    axis=mybir.AxisListType.X,
                                    op=mybir.AluOpType.add)
            sq64 = sb.tile([64, 2], F32)
            nc.vector.tensor_copy(sq64[:, 0:1], s128[0:64, :])
            nc.sync.dma_start(sq64[:, 1:2], s128[64:128, :])

            # ---- AllReduce over cores ----
            cc_in = dram.tile([64, 2], F32)
            cc_out = dram.tile([64, 2], F32)
            nc.gpsimd.dma_start(cc_in[:], sq64[:])
            nc.gpsimd.collective_compute(
                "AllReduce", mybir.AluOpType.add,
                replica_groups=[list(range(C))],
                ins=[cc_in.opt()], outs=[cc_out.opt()])
            g2 = sb.tile([64, 2], F32)
            nc.sync.dma_start(g2[:], cc_out[:])
            # per-channel BN coefficients: col0 = gamma/std, col1 = beta - mean*that
            me = sb.tile([64, 2], F32)
            nc.vector.tensor_scalar_mul(me[:], g2[:], 1.0 / N)  # [mean, Eh2]
            v1 = sb.tile([64, 1], F32)
            nc.vector.tensor_tensor(out=v1[:], in0=me[:, 0:1], in1=me[:, 0:1],
                                    op=mybir.AluOpType.mult)
            nc.vector.tensor_tensor(out=v1[:], in0=me[:, 1:2], in1=v1[:],
                                    op=mybir.AluOpType.subtract)
            eps_t = sb.tile([64, 1], F32)
            nc.gpsimd.memset(eps_t[:], EPS)
            std = sb.tile([64, 1], F32)
            nc.scalar.activation(std[:], v1[:], mybir.ActivationFunctionType.Sqrt,
                                 bias=eps_t[:])
            rin = sb.tile([64, 1], F32)
            nc.vector.reciprocal(rin[:], std[:])
            gam = sb.tile([64, 1], F32)
            nc.sync.dma_start(gam[:], gam_d[0, :, None])
            bet = sb.tile([64, 1], F32)
            nc.sync.dma_start(bet[:], bet_d[0, :, None])
            sc_h = sb.tile([64, 2], F32)
            nc.vector.tensor_tensor(out=sc_h[:, 0:1], in0=rin[:], in1=gam[:],
                                    op=mybir.AluOpType.mult)
            nc.vector.tensor_tensor(out=sc_h[:, 1:2], in0=me[:, 0:1],
                                    in1=sc_h[:, 0:1],
                                    op=mybir.AluOpType.mult)
            nc.vector.tensor_tensor(out=sc_h[:, 1:2], in0=bet[:],
                                    in1=sc_h[:, 1:2],
                                    op=mybir.AluOpType.subtract)

            # ---- broadcast scale/bias to [128, 512] column tiles ----
            pt2 = ps.tile([128, 128], F32, tag="psT", space="PSUM")
            nc.tensor.transpose(out=pt2[0:1, 0:64], in_=sc_h[:, 0:1],
                                identity=ident[0:64, 0:64])
            nc.tensor.transpose(out=pt2[0:1, 64:128], in_=sc_h[:, 1:2],
                                identity=ident[0:64, 0:64])
            srow_s = sb.tile([1, 512], F32)
            srow_b = sb.tile([1, 512], F32)
            for m in range(8):
                nc.vector.tensor_copy(srow_s[:, m * 64:(m + 1) * 64],
                                      pt2[0:1, 0:64])
                nc.vector.tensor_copy(srow_b[:, m * 64:(m + 1) * 64],
                                      pt2[0:1, 64:128])
            ones1 = sb.tile([1, 128], F32)
            nc.gpsimd.memset(ones1[:], 1.0)
            pbc = ps.tile([128, 512], F32, tag="psB", space="PSUM")
            nc.tensor.matmul(out=pbc[:], lhsT=ones1[:], rhs=srow_s[:],
                             start=True, stop=True)
            scale_bc = sb.tile([128, 512], F32)
            nc.vector.tensor_copy(scale_bc[:], pbc[:])
            pbc2 = ps.tile([128, 512], F32, tag="psB", space="PSUM")
            nc.tensor.matmul(out=pbc2[:], lhsT=ones1[:], rhs=srow_b[:],
                             start=True, stop=True)
            bias_bc = sb.tile([128, 512], F32)
            nc.vector.tensor_copy(bias_bc[:], pbc2[:])

            # ---- apply Lrelu(h*scale + bias) in place, then write y ----
            for b, off, wdt in slices:
                t = io.tile([128, 512], F32, tag="ap")
                nc.vector.tensor_tensor(out=t[:, 0:wdt], in0=b[:, off:off + wdt],
                                        in1=scale_bc[:, 0:wdt],
                                        op=mybir.AluOpType.mult)
                nc.vector.tensor_tensor(out=t[:, 0:wdt], in0=t[:, 0:wdt],
                                        in1=bias_bc[:, 0:wdt],
                                        op=mybir.AluOpType.add)
                nc.scalar.activation(b[:, off:off + wdt], t[:, 0:wdt],
                                     mybir.ActivationFunctionType.Lrelu,
                                     bias=0.0, alpha=NEG)

            # y rows: block = 2g+r -> buf r, pair col g
            yv = y_d[0:24832, :].rearrange("(g two p) c -> two p g c",
                                           two=2, p=128)
            nc.sync.dma_start(yv[0], hA[:, 0:97 * D].rearrange(
                "p (g c) -> p g c", c=D))
            nc.sync.dma_start(yv[1], hB[:, 0:97 * D].rearrange(
                "p (g c) -> p g c", c=D))
            nc.sync.dma_start(y_d[24832:24960, :], hA[:, 97 * D:98 * D])
            nc.sync.dma_start(y_d[24960:25000, :], hB[0:40, 97 * D:98 * D])


_CACHE = {}
_CURRENT_PLAN = None


def build(nbr):
    key = nbr.tobytes()[:4096] + nbr.tobytes()[-4096:]
    if key in _CACHE:
        return _CACHE[key]
    plan, NK, GT, gslab, sslab = _prep_host(np.asarray(nbr, np.int64))
    NKmax = max(NK.values())
    global _CURRENT_PLAN
    _CURRENT_PLAN = (plan, GT, NKmax)
    nc = _build_program(plan, GT, NKmax)
    _CACHE[key] = (nc, gslab, sslab)
    return nc, gslab, sslab


def kernel(x, W, gamma, beta, nbr):
    x = np.ascontiguousarray(np.asarray(x, np.float32))
    W = np.ascontiguousarray(np.asarray(W, np.float32))
    gamma = np.asarray(gamma, np.float32).reshape(1, D)
    beta = np.asarray(beta, np.float32).reshape(1, D)
    nbr = np.asarray(nbr)
    nc, gslab, sslab = build(nbr)
    in_maps = []
    for c in range(C):
        in_maps.append({
            "x_d": x,
            "xc_d": x[c * V:(c + 1) * V],
            "W_d": W,
            "gam_d": gamma,
            "bet_d": beta,
            "gi_d": gslab[c],
            "si_d": sslab[c],
        })
    res = bass_utils.run_bass_kernel_spmd(nc, in_maps, core_ids=list(range(C)))
    return np.concatenate([res.results[c]["y_d"] for c in range(C)], axis=0)
